# revision 11
# baseline (speedup 1.0000x reference)
"""Trainium2 Bass kernel for a 2-layer GATv2 (DGL-style) over a random graph.

Self-contained: takes FULL inputs (as produced by the problem's setup_inputs),
shards across 8 NeuronCores internally, returns the FULL output [N, 64] f32.

Strategy (per core, dst-sharded, degree-sorted rep-major edge layout):
 - Each core owns N/8 dst nodes and all edges pointing to them.
 - Dst nodes are sorted by in-degree and grouped 128 at a time; each group is
   padded to its max degree d̂_g (cross-core max).  Edge slots are laid out
   rep-major: slot = (tile r, lane) holds edge #r of the dst at lane.  Every
   128-edge tile therefore has dst == partition index, so fd[dst] is just the
   group's fd tile (no per-edge fd gather) and the segment-sum is a plain
   per-partition accumulation (no one-hot scatter matmuls).
 - Only fs[src] needs a true per-edge gather.  dma_gather's int16 indices
   can't address 50k rows, so tables are stored as 512-byte pair rows
   [row(2j) | row(2j+1)] and the half is selected on-device by parity masks.
 - Node tables are built by on-device matmuls; fs tables are AllGather'd so
   each core can gather any src row.  The dst permutation is undone on the
   host when assembling the output.
"""

import math

import numpy as np
import ml_dtypes

import concourse.bass as bass
import concourse.mybir as mybir
import concourse.tile as tile
from concourse import bacc
from concourse._compat import axon_active

P = 128
F32 = mybir.dt.float32
BF16 = mybir.dt.bfloat16
I16 = mybir.dt.int16

NEG_SLOPE = 0.2
DEN_EPS = 1e-20
CH = 8            # tiles per DVE chunk
SG_TILES = 48     # max tiles per gather batch (SBUF budget)
MAXI = 1024       # max indices per dma_gather (HW desc-ring limit)


class Cfg:
    def __init__(self, N=50000, E=800000, F_IN=128, H1=4, D1=32, OUT=64, NC=8):
        self.N, self.E, self.F_IN, self.H1, self.D1, self.OUT, self.NC = \
            N, E, F_IN, H1, D1, OUT, NC
        self.D1TOT = H1 * D1              # 128
        self.NLOC = N // NC               # 6250
        self.NG = math.ceil(self.NLOC / P)
        self.NPAD = self.NG * P           # 6272
        self.N8 = self.NPAD * NC          # 50176 (permuted-global table rows)
        # filled by preprocess:
        self.NT = None                    # [NG] tiles per group (cross-core max)
        self.offs = None                  # [NG+1] tile offsets
        self.TT = None                    # total tiles


def _wrap_idx(arr_i16):
    """[n] int16 -> [128, n/16] idx tile layout (16-partition wrap, 8x rep)."""
    n = arr_i16.shape[0]
    assert n % 16 == 0
    idx16 = arr_i16.reshape(-1, 16).T           # [16, n/16]
    return np.tile(idx16, (8, 1)).copy()        # [128, n/16]


def _slot_cols(arr, dtype):
    """[n*128] -> [128, n]: slot s -> partition s%128, col s//128."""
    return np.ascontiguousarray(arr.reshape(-1, P).T).astype(dtype)


def preprocess(inputs, cfg: Cfg):
    x = np.asarray(inputs["x"], np.float32)
    src = np.asarray(inputs["src"], np.int64)
    dst = np.asarray(inputs["dst"], np.int64)
    N, NC, NLOC, NG, NPAD = cfg.N, cfg.NC, cfg.NLOC, cfg.NG, cfg.NPAD

    # per-core degree sort
    cores = []
    pgid = np.empty(N, np.int64)          # original global id -> permuted gid
    for c in range(NC):
        m = (dst >= c * NLOC) & (dst < (c + 1) * NLOC)
        es, ed = src[m], dst[m] - c * NLOC
        deg = np.bincount(ed, minlength=NLOC)
        order = np.argsort(-deg, kind="stable")       # sorted pos -> local id
        pos_of = np.empty(NLOC, np.int64)
        pos_of[order] = np.arange(NLOC)
        pgid[c * NLOC:(c + 1) * NLOC] = c * NPAD + pos_of
        degs = np.concatenate([deg[order], np.zeros(NPAD - NLOC, np.int64)])
        gmax = degs.reshape(NG, P).max(axis=1)
        cores.append((es, ed, deg, order, pos_of, gmax))

    NT = np.zeros(NG, np.int64)
    for (_, _, _, _, _, gmax) in cores:
        NT = np.maximum(NT, gmax)
    NT = np.maximum(NT, 1)
    cfg.NT = NT.tolist()
    cfg.offs = np.concatenate([[0], np.cumsum(NT)]).tolist()
    cfg.TT = int(NT.sum())
    TT = cfg.TT

    # weights
    Wl1 = np.asarray(inputs["Wl1"], np.float32)
    Wr1 = np.asarray(inputs["Wr1"], np.float32)
    bl1 = np.asarray(inputs["bl1"], np.float32)
    br1 = np.asarray(inputs["br1"], np.float32)
    attn1 = np.asarray(inputs["attn1"], np.float32)
    Wl2 = np.asarray(inputs["Wl2"], np.float32)
    Wr2 = np.asarray(inputs["Wr2"], np.float32)
    bl2 = np.asarray(inputs["bl2"], np.float32)
    br2 = np.asarray(inputs["br2"], np.float32)
    attn2 = np.asarray(inputs["attn2"], np.float32)

    W1cat = np.concatenate([Wl1, Wr1], axis=1)                  # [128, 256] f32
    b1cat = np.zeros((1, 256), np.float32)
    b1cat[0, 128:256] = bl1 + br1          # fd side carries both linear biases
    W2cat = np.concatenate([Wl2, Wr2], axis=1).astype(ml_dtypes.bfloat16)
    b2cat = np.zeros((1, 128), np.float32)
    b2cat[0, 64:128] = bl2 + br2
    b2cat = b2cat.astype(ml_dtypes.bfloat16)
    attn1_rep = np.tile(attn1.reshape(1, -1), (P, 1)).astype(ml_dtypes.bfloat16)
    attn2_rep = np.tile(attn2.reshape(1, -1), (P, 1)).astype(ml_dtypes.bfloat16)
    bl1_rep = np.tile(bl1.reshape(1, -1), (P, 1)).astype(ml_dtypes.bfloat16)
    ident_bf = np.eye(P, dtype=ml_dtypes.bfloat16)
    ones_f32 = np.ones((1, P), np.float32)
    ones_bf = np.ones((1, P), ml_dtypes.bfloat16)

    in_maps = []
    deg_order = []
    for c in range(NC):
        es, ed, deg, order, pos_of, _ = cores[c]
        # slot assignment: edge -> (group, rep, lane)
        p_e = pos_of[ed]                        # sorted position of dst
        g_e = p_e // P
        lane = p_e % P
        o = np.argsort(p_e, kind="stable")
        p_s, g_s, lane_s, es_s = p_e[o], g_e[o], lane[o], es[o]
        # rank within dst
        cnt = np.bincount(p_s, minlength=NPAD)
        run_start = np.concatenate([[0], np.cumsum(cnt)[:-1]])
        rank = np.arange(es_s.shape[0]) - run_start[p_s]
        tile_i = np.asarray(cfg.offs)[g_s] + rank
        slot = tile_i * P + lane_s

        pg = pgid[es_s]
        idx = np.zeros(TT * P, np.int64)
        m0 = np.zeros(TT * P, np.float32)
        m1 = np.zeros(TT * P, np.float32)
        msk = np.zeros(TT * P, np.float32)
        idx[slot] = pg // 2
        par = (pg & 1).astype(np.float32)
        m0[slot] = 1.0 - par
        m1[slot] = par
        msk[slot] = 1.0

        # permuted, transposed node features
        xT = np.zeros((cfg.F_IN, NPAD), np.float32)
        xT[:, :NLOC] = x[c * NLOC:(c + 1) * NLOC][order].T

        in_maps.append({
            "xT": xT,
            "W1cat": W1cat, "b1cat": b1cat,
            "W2cat": np.asarray(W2cat), "b2cat": np.asarray(b2cat),
            "attn1_rep": np.asarray(attn1_rep),
            "attn2_rep": np.asarray(attn2_rep),
            "bl1_rep": np.asarray(bl1_rep),
            "ident_bf": np.asarray(ident_bf),
            "ones_f32": ones_f32, "ones_bf": np.asarray(ones_bf),
            "idx": _wrap_idx(idx.astype(np.int16)),
            "m1i": _slot_cols(m1, np.int8),
            "maskt": _slot_cols(msk, ml_dtypes.bfloat16),
        })
        deg_order.append((deg, order))
    cfg._deg_order = deg_order
    cfg._bl2 = bl2
    return in_maps


def _gather_batches(cfg):
    """Partition groups into consecutive batches of <= SG_TILES tiles."""
    batches = []
    cur = []
    cur_t = 0
    for g in range(cfg.NG):
        t = cfg.NT[g]
        if cur and cur_t + t > SG_TILES:
            batches.append(cur)
            cur, cur_t = [], 0
        cur.append(g)
        cur_t += t
    if cur:
        batches.append(cur)
    return batches


def build_program(cfg: Cfg, debug=False):
    nc = bacc.Bacc("TRN2", target_bir_lowering=False, debug=debug,
                   num_devices=cfg.NC)
    NG, NPAD, NLOC, TT = cfg.NG, cfg.NPAD, cfg.NLOC, cfg.TT
    F_IN, D1TOT, H1, D1, OUT = cfg.F_IN, cfg.D1TOT, cfg.H1, cfg.D1, cfg.OUT
    offs = cfg.offs
    core_ids = list(range(cfg.NC))

    par = {}

    def param(name, shape, dtype):
        par[name] = nc.declare_dram_parameter(name, list(shape), dtype,
                                              isOutput=False)
        return par[name]

    xT = param("xT", (F_IN, NPAD), F32)
    W1cat = param("W1cat", (F_IN, 2 * D1TOT), F32)
    b1cat = param("b1cat", (1, 2 * D1TOT), F32)
    W2cat = param("W2cat", (D1TOT, 2 * OUT), BF16)
    b2cat = param("b2cat", (1, 2 * OUT), BF16)
    attn1_rep = param("attn1_rep", (P, D1TOT), BF16)
    attn2_rep = param("attn2_rep", (P, OUT), BF16)
    bl1_rep = param("bl1_rep", (P, D1TOT), BF16)
    ident_bf = param("ident_bf", (P, P), BF16)
    ones_f32 = param("ones_f32", (1, P), F32)
    ones_bf = param("ones_bf", (1, P), BF16)
    idx = param("idx", (P, TT * 8), I16)
    m1i = param("m1i", (P, TT), mybir.dt.int8)
    maskt = param("maskt", (P, TT), BF16)

    out_local = nc.declare_dram_parameter("out_local", [NPAD, OUT], F32,
                                          isOutput=True)

    fs1_local = nc.dram_tensor("fs1_local", [NPAD, D1TOT], BF16)
    fs1_full = nc.dram_tensor("fs1_full", [cfg.N8 // 2, 2 * D1TOT], BF16,
                              addr_space="Shared")
    fs2_local = nc.dram_tensor("fs2_local", [NPAD, 2 * OUT], BF16)
    fs2_full = nc.dram_tensor("fs2_full", [cfg.N8 // 2, 4 * OUT], BF16,
                              addr_space="Shared")

    batches = _gather_batches(cfg)

    with tile.TileContext(nc) as tc:
        with tc.tile_pool(name="const", bufs=1) as cpool:
            c_attn1 = cpool.tile([P, D1TOT], BF16)
            nc.sync.dma_start(out=c_attn1[:], in_=attn1_rep[:, :])
            c_attn2 = cpool.tile([P, OUT], BF16)
            nc.sync.dma_start(out=c_attn2[:], in_=attn2_rep[:, :])
            c_bl1 = cpool.tile([P, D1TOT], BF16)
            nc.sync.dma_start(out=c_bl1[:], in_=bl1_rep[:, :])
            c_ident = cpool.tile([P, P], BF16)
            nc.sync.dma_start(out=c_ident[:], in_=ident_bf[:, :])
            c_ones_f = cpool.tile([1, P], F32)
            nc.sync.dma_start(out=c_ones_f[:], in_=ones_f32[:, :])
            c_ones_bf = cpool.tile([1, P], BF16)
            nc.sync.dma_start(out=c_ones_bf[:], in_=ones_bf[:, :])
            c_W2 = cpool.tile([D1TOT, 2 * OUT], BF16)
            nc.sync.dma_start(out=c_W2[:], in_=W2cat[:, :])
            c_b2 = cpool.tile([1, 2 * OUT], BF16)
            nc.sync.dma_start(out=c_b2[:], in_=b2cat[:, :])
            c_idx = cpool.tile([P, TT * 8], I16)
            nc.sync.dma_start(out=c_idx[:], in_=idx[:, :])
            c_m1 = cpool.tile([P, TT], mybir.dt.int8)
            nc.sync.dma_start(out=c_m1[:], in_=m1i[:, :])
            c_msk = cpool.tile([P, TT], BF16)
            nc.sync.dma_start(out=c_msk[:], in_=maskt[:, :])
            c_fd1 = cpool.tile([P, NG * D1TOT], BF16)
            c_fd2 = cpool.tile([P, NG * OUT], BF16)

            # ================= phase A: layer-1 node tables =================
            with (
                tc.tile_pool(name="ph0", bufs=1) as p0,
                tc.tile_pool(name="ph0w", bufs=3) as p0w,
                tc.tile_pool(name="psA", bufs=2, space="PSUM") as psA,
            ):
                c_W1 = p0.tile([F_IN, 2 * D1TOT], F32)
                nc.sync.dma_start(out=c_W1[:], in_=W1cat[:, :])
                c_b1 = p0.tile([1, 2 * D1TOT], F32)
                nc.sync.dma_start(out=c_b1[:], in_=b1cat[:, :])
                c_xT = p0.tile([F_IN, NPAD], F32)
                nc.sync.dma_start(out=c_xT[:], in_=xT[:, :])
                for g in range(NG):
                    ps = psA.tile([P, 2 * D1TOT], F32)
                    nc.tensor.matmul(out=ps[:], lhsT=c_xT[:, g * P:(g + 1) * P],
                                     rhs=c_W1[:], start=True, stop=False)
                    nc.tensor.matmul(out=ps[:], lhsT=c_ones_f[:], rhs=c_b1[:],
                                     start=False, stop=True)
                    sb = p0w.tile([P, D1TOT], BF16, tag="t1sb")
                    nc.scalar.activation(
                        out=sb[:], in_=ps[:, 0:D1TOT],
                        func=mybir.ActivationFunctionType.Copy)
                    nc.scalar.activation(
                        out=c_fd1[:, g * D1TOT:(g + 1) * D1TOT],
                        in_=ps[:, D1TOT:2 * D1TOT],
                        func=mybir.ActivationFunctionType.Copy)
                    nc.sync.dma_start(out=fs1_local[g * P:(g + 1) * P, :],
                                      in_=sb[:])

            tc.strict_bb_all_engine_barrier()
            nc.gpsimd.collective_compute(
                "AllGather", mybir.AluOpType.bypass,
                replica_groups=[core_ids],
                ins=[fs1_local[:, :]], outs=[fs1_full[:, :]],
            )
            tc.strict_bb_all_engine_barrier()

            # ============== phase B: layer-1 edges + layer-2 tables =========
            _edge_phase(
                nc, tc, cfg, layer=1, table=fs1_full,
                c_idx=c_idx, c_m1=c_m1, c_msk=c_msk,
                c_fd=c_fd1, c_attn=c_attn1, feat=D1TOT, nheads=H1, hdim=D1,
                batches=batches,
                c_bl1=c_bl1, c_ident=c_ident, c_W2=c_W2, c_b2=c_b2,
                c_ones_bf=c_ones_bf, c_fd2=c_fd2,
                fs2_local=fs2_local, out_local=None,
            )

            tc.strict_bb_all_engine_barrier()
            nc.gpsimd.collective_compute(
                "AllGather", mybir.AluOpType.bypass,
                replica_groups=[core_ids],
                ins=[fs2_local[:, :]], outs=[fs2_full[:, :]],
            )
            tc.strict_bb_all_engine_barrier()

            # ================= phase D: layer-2 edges =======================
            _edge_phase(
                nc, tc, cfg, layer=2, table=fs2_full,
                c_idx=c_idx, c_m1=c_m1, c_msk=c_msk,
                c_fd=c_fd2, c_attn=c_attn2, feat=OUT, nheads=1, hdim=OUT,
                batches=batches,
                c_bl1=None, c_ident=None, c_W2=None, c_b2=None,
                c_ones_bf=None, c_fd2=None,
                fs2_local=None, out_local=out_local,
            )

    nc.compile()
    return nc


def _edge_phase(nc, tc, cfg, layer, table,
                c_idx, c_m1, c_msk, c_fd, c_attn,
                feat, nheads, hdim, batches,
                c_bl1, c_ident, c_W2, c_b2, c_ones_bf, c_fd2,
                fs2_local, out_local):
    offs = cfg.offs
    OUT = cfg.OUT
    acols = feat + nheads
    # pair-row layout: row j of `table` = [node(2j) | node(2j+1)], each `row_w`
    # wide, of which the first `feat` cols are the fs features we use.
    row_w = cfg.D1TOT if layer == 1 else 2 * OUT

    def gather_chunked(out_tile, idx_col0, n_idx):
        done = 0
        while done < n_idx:
            n = min(MAXI, n_idx - done)
            nc.gpsimd.dma_gather(
                out_ap=out_tile[:, (done // P):((done + n) // P), :],
                in_ap=table[:, :],
                idxs_ap=c_idx[:, idx_col0 + done // 16:idx_col0 + (done + n) // 16],
                num_idxs=n, num_idxs_reg=n, elem_size=2 * row_w)
            done += n

    NTMAX = max(cfg.NT)
    with (
        tc.tile_pool(name=f"g{layer}", bufs=2) as gp,
        tc.tile_pool(name=f"wk{layer}", bufs=3) as wk,
        tc.tile_pool(name=f"sm{layer}", bufs=4) as sm,
        tc.tile_pool(name=f"ac{layer}", bufs=2) as ap_,
        tc.tile_pool(name=f"ps{layer}", bufs=2, space="PSUM") as pp,
    ):
        for batch in batches:
            t0, t1_ = offs[batch[0]], offs[batch[-1] + 1]
            nt_b = t1_ - t0
            gbuf = gp.tile([P, nt_b, 2 * row_w], BF16, tag="g")
            gather_chunked(gbuf, t0 * 8, nt_b * P)

            fsb = gp.tile([P, nt_b, feat], BF16, tag="fsb")
            nc.vector.tensor_copy(out=fsb[:], in_=gbuf[:, :, 0:feat])
            nc.vector.copy_predicated(
                out=fsb[:],
                mask=c_m1[:, t0:t1_].unsqueeze(2).to_broadcast(
                    [P, nt_b, feat]),
                data=gbuf[:, :, row_w:row_w + feat])

            for g in batch:
                ng_t = offs[g + 1] - offs[g]
                wbuf = ap_.tile([P, NTMAX, feat], BF16, tag="wb")
                lgb = ap_.tile([P, NTMAX * nheads], F32, tag="lgb")
                fd_g = c_fd[:, g * feat:(g + 1) * feat]
                for c0 in range(offs[g], offs[g + 1], CH):
                    nt = min(CH, offs[g + 1] - c0)
                    w0 = c0 - offs[g]
                    fs = fsb[:, c0 - t0:c0 - t0 + nt, :]
                    u = wk.tile([P, CH, feat], BF16, tag="u")
                    nc.vector.tensor_tensor(
                        out=u[:, 0:nt, :], in0=fs,
                        in1=fd_g.unsqueeze(1).to_broadcast([P, nt, feat]),
                        op=mybir.AluOpType.add)
                    t_lr = wk.tile([P, CH, feat], BF16, tag="tlr")
                    uf = u[:, 0:nt, :]
                    nc.vector.scalar_tensor_tensor(
                        out=t_lr[:, 0:nt, :], in0=uf, scalar=NEG_SLOPE,
                        in1=uf, op0=mybir.AluOpType.mult,
                        op1=mybir.AluOpType.max)
                    tp = wk.tile([P, CH, feat], BF16, tag="tp")
                    nc.vector.tensor_tensor(
                        out=tp[:, 0:nt, :], in0=t_lr[:, 0:nt, :],
                        in1=c_attn[:].unsqueeze(1).to_broadcast([P, nt, feat]),
                        op=mybir.AluOpType.mult)
                    nc.vector.tensor_reduce(
                        out=lgb[:, w0 * nheads:(w0 + nt) * nheads].rearrange(
                            "p (n h) -> p n h", n=nt),
                        in_=tp[:, 0:nt, :].rearrange(
                            "p n (h d) -> p n h d", h=nheads),
                        axis=mybir.AxisListType.X, op=mybir.AluOpType.add)

                # ---- per-group: softmax numerators + reduce + normalize ----
                a_g = sm.tile([P, NTMAX * nheads], BF16, tag="a")
                nc.scalar.activation(
                    out=a_g[:, 0:ng_t * nheads], in_=lgb[:, 0:ng_t * nheads],
                    func=mybir.ActivationFunctionType.Exp)
                am_g = sm.tile([P, NTMAX * nheads], BF16, tag="am")
                nc.vector.tensor_tensor(
                    out=am_g[:, 0:ng_t * nheads].rearrange(
                        "p (n h) -> p n h", n=ng_t),
                    in0=a_g[:, 0:ng_t * nheads].rearrange(
                        "p (n h) -> p n h", n=ng_t),
                    in1=c_msk[:, offs[g]:offs[g + 1]].unsqueeze(2)
                        .to_broadcast([P, ng_t, nheads]),
                    op=mybir.AluOpType.mult)
                nc.vector.tensor_tensor(
                    out=wbuf[:, 0:ng_t, :].rearrange(
                        "p n (h d) -> p n h d", h=nheads),
                    in0=fsb[:, offs[g] - t0:offs[g] - t0 + ng_t, :].rearrange(
                        "p n (h d) -> p n h d", h=nheads),
                    in1=am_g[:, 0:ng_t * nheads].rearrange(
                        "p (n h) -> p n h", n=ng_t).unsqueeze(3)
                        .to_broadcast([P, ng_t, nheads, hdim]),
                    op=mybir.AluOpType.mult)
                acc = sm.tile([P, acols], F32, tag="acc")
                nc.vector.tensor_reduce(
                    out=acc[:, 0:feat],
                    in_=wbuf[:, 0:ng_t, :].rearrange("p n f -> p f n"),
                    axis=mybir.AxisListType.X, op=mybir.AluOpType.add)
                nc.vector.tensor_reduce(
                    out=acc[:, feat:acols],
                    in_=am_g[:, 0:ng_t * nheads].rearrange(
                        "p (n h) -> p h n", n=ng_t),
                    axis=mybir.AxisListType.X, op=mybir.AluOpType.add)
                den0 = acc[:, feat:acols]
                if layer == 1:
                    # fold bl1 back in: acc_w += den0 * bl1 (per head)
                    bt = sm.tile([P, feat], F32, tag="bt")
                    nc.vector.tensor_tensor(
                        out=bt[:].rearrange("p (h d) -> p h d", h=nheads),
                        in0=c_bl1[:].rearrange("p (h d) -> p h d", h=nheads),
                        in1=den0.unsqueeze(2).to_broadcast([P, nheads, hdim]),
                        op=mybir.AluOpType.mult)
                    nc.vector.tensor_tensor(
                        out=acc[:, 0:feat], in0=acc[:, 0:feat], in1=bt[:],
                        op=mybir.AluOpType.add)
                den = sm.tile([P, nheads], F32, tag="den")
                nc.vector.tensor_scalar_add(
                    out=den[:], in0=den0, scalar1=DEN_EPS)
                denr = sm.tile([P, nheads], F32, tag="denr")
                nc.vector.reciprocal(out=denr[:], in_=den[:])
                if layer == 1:
                    h_g = wk.tile([P, feat], BF16, tag="hg")
                    nc.vector.scalar_tensor_tensor(
                        out=h_g[:].rearrange("p (h d) -> p h d", h=nheads),
                        in0=acc[:, 0:feat].rearrange("p (h d) -> p h d",
                                                     h=nheads),
                        scalar=0.0, op0=mybir.AluOpType.max,
                        in1=denr[:].unsqueeze(2).to_broadcast(
                            [P, nheads, hdim]),
                        op1=mybir.AluOpType.mult)
                    ps_t = pp.tile([P, P], BF16, tag="pst")
                    nc.tensor.transpose(out=ps_t[:], in_=h_g[:],
                                        identity=c_ident[:])
                    hT = wk.tile([P, P], BF16, tag="hT")
                    nc.scalar.activation(
                        out=hT[:], in_=ps_t[:],
                        func=mybir.ActivationFunctionType.Copy)
                    ps2 = pp.tile([P, 2 * OUT], F32, tag="ps2")
                    nc.tensor.matmul(out=ps2[:], lhsT=hT[:], rhs=c_W2[:],
                                     start=True, stop=False)
                    nc.tensor.matmul(out=ps2[:], lhsT=c_ones_bf[:],
                                     rhs=c_b2[:], start=False, stop=True)
                    sb2 = wk.tile([P, 2 * OUT], BF16, tag="sb2")
                    nc.scalar.activation(
                        out=sb2[:], in_=ps2[:],
                        func=mybir.ActivationFunctionType.Copy)
                    nc.scalar.activation(
                        out=c_fd2[:, g * OUT:(g + 1) * OUT],
                        in_=sb2[:, OUT:2 * OUT],
                        func=mybir.ActivationFunctionType.Copy)
                    nc.sync.dma_start(out=fs2_local[g * P:(g + 1) * P, :],
                                      in_=sb2[:])
                else:
                    o_g = wk.tile([P, feat], F32, tag="og")
                    nc.vector.tensor_tensor(
                        out=o_g[:], in0=acc[:, 0:feat],
                        in1=denr[:].to_broadcast([P, feat]),
                        op=mybir.AluOpType.mult)
                    nc.sync.dma_start(out=out_local[g * P:(g + 1) * P, :],
                                      in_=o_g[:])


def assemble(results, cfg: Cfg):
    out = np.empty((cfg.N, cfg.OUT), np.float32)
    bl2 = cfg._bl2
    for c in range(cfg.NC):
        loc = np.asarray(results[c]["out_local"])[:cfg.NLOC]   # sorted order
        deg, order = cfg._deg_order[c]
        blk = np.empty((cfg.NLOC, cfg.OUT), np.float32)
        blk[order] = loc
        blk[deg > 0] += bl2[None, :]
        out[c * cfg.NLOC:(c + 1) * cfg.NLOC] = blk
    return out


def kernel(**inputs):
    from concourse.bass_utils import run_bass_kernel_spmd
    cfg = Cfg()
    in_maps = preprocess(inputs, cfg)
    nc = build_program(cfg, debug=not axon_active())
    res = run_bass_kernel_spmd(nc, in_maps, list(range(cfg.NC)))
    return assemble(res.results, cfg)


# revision 12
# speedup vs baseline: 1.1036x; 1.1036x over previous
"""Trainium2 Bass kernel for a 2-layer GATv2 (DGL-style) over a random graph.

Self-contained: takes FULL inputs (as produced by the problem's setup_inputs),
shards across 8 NeuronCores internally, returns the FULL output [N, 64] f32.

Strategy (per core, dst-sharded, degree-sorted rep-major edge layout):
 - Each core owns N/8 dst nodes and all edges pointing to them.
 - Dst nodes are sorted by in-degree and grouped 128 at a time; each group is
   padded to its max degree d̂_g (cross-core max).  Edge slots are laid out
   rep-major: slot = (tile r, lane) holds edge #r of the dst at lane.  Every
   128-edge tile therefore has dst == partition index, so fd[dst] is just the
   group's fd tile (no per-edge fd gather) and the segment-sum is a plain
   per-partition accumulation (no one-hot scatter matmuls).
 - Only fs[src] needs a true per-edge gather.  dma_gather's int16 indices
   can't address 50k rows, so tables are stored as 512-byte pair rows
   [row(2j) | row(2j+1)] and the half is selected on-device by parity masks.
 - Node tables are built by on-device matmuls; fs tables are AllGather'd so
   each core can gather any src row.  The dst permutation is undone on the
   host when assembling the output.
"""

import math

import numpy as np
import ml_dtypes

import concourse.bass as bass
import concourse.mybir as mybir
import concourse.tile as tile
from concourse import bacc
from concourse._compat import axon_active

P = 128
F32 = mybir.dt.float32
BF16 = mybir.dt.bfloat16
I16 = mybir.dt.int16

NEG_SLOPE = 0.2
DEN_EPS = 1e-20
CH = 8            # tiles per DVE chunk
SG_TILES = 48     # max tiles per gather batch (SBUF budget)
MAXI = 1024       # max indices per dma_gather (HW desc-ring limit)


class Cfg:
    def __init__(self, N=50000, E=800000, F_IN=128, H1=4, D1=32, OUT=64, NC=8):
        self.N, self.E, self.F_IN, self.H1, self.D1, self.OUT, self.NC = \
            N, E, F_IN, H1, D1, OUT, NC
        self.D1TOT = H1 * D1              # 128
        self.NLOC = N // NC               # 6250
        self.NG = math.ceil(self.NLOC / P)
        self.NPAD = self.NG * P           # 6272
        self.N8 = self.NPAD * NC          # 50176 (permuted-global table rows)
        # filled by preprocess:
        self.NT = None                    # [NG] tiles per group (cross-core max)
        self.offs = None                  # [NG+1] tile offsets
        self.TT = None                    # total tiles


def _wrap_idx(arr_i16):
    """[n] int16 -> [128, n/16] idx tile layout (16-partition wrap, 8x rep)."""
    n = arr_i16.shape[0]
    assert n % 16 == 0
    idx16 = arr_i16.reshape(-1, 16).T           # [16, n/16]
    return np.tile(idx16, (8, 1)).copy()        # [128, n/16]


def _slot_cols(arr, dtype):
    """[n*128] -> [128, n]: slot s -> partition s%128, col s//128."""
    return np.ascontiguousarray(arr.reshape(-1, P).T).astype(dtype)


def preprocess(inputs, cfg: Cfg):
    x = np.asarray(inputs["x"], np.float32)
    src = np.asarray(inputs["src"], np.int64)
    dst = np.asarray(inputs["dst"], np.int64)
    N, NC, NLOC, NG, NPAD = cfg.N, cfg.NC, cfg.NLOC, cfg.NG, cfg.NPAD

    # per-core degree sort
    cores = []
    pgid = np.empty(N, np.int64)          # original global id -> permuted gid
    for c in range(NC):
        m = (dst >= c * NLOC) & (dst < (c + 1) * NLOC)
        es, ed = src[m], dst[m] - c * NLOC
        deg = np.bincount(ed, minlength=NLOC)
        order = np.argsort(-deg, kind="stable")       # sorted pos -> local id
        pos_of = np.empty(NLOC, np.int64)
        pos_of[order] = np.arange(NLOC)
        pgid[c * NLOC:(c + 1) * NLOC] = c * NPAD + pos_of
        degs = np.concatenate([deg[order], np.zeros(NPAD - NLOC, np.int64)])
        gmax = degs.reshape(NG, P).max(axis=1)
        cores.append((es, ed, deg, order, pos_of, gmax))

    NT = np.zeros(NG, np.int64)
    for (_, _, _, _, _, gmax) in cores:
        NT = np.maximum(NT, gmax)
    NT = np.maximum(NT, 1)
    cfg.NT = NT.tolist()
    cfg.offs = np.concatenate([[0], np.cumsum(NT)]).tolist()
    cfg.TT = int(NT.sum())
    TT = cfg.TT

    # weights
    Wl1 = np.asarray(inputs["Wl1"], np.float32)
    Wr1 = np.asarray(inputs["Wr1"], np.float32)
    bl1 = np.asarray(inputs["bl1"], np.float32)
    br1 = np.asarray(inputs["br1"], np.float32)
    attn1 = np.asarray(inputs["attn1"], np.float32)
    Wl2 = np.asarray(inputs["Wl2"], np.float32)
    Wr2 = np.asarray(inputs["Wr2"], np.float32)
    bl2 = np.asarray(inputs["bl2"], np.float32)
    br2 = np.asarray(inputs["br2"], np.float32)
    attn2 = np.asarray(inputs["attn2"], np.float32)

    W1cat = np.concatenate([Wl1, Wr1], axis=1)                  # [128, 256] f32
    b1cat = np.zeros((1, 256), np.float32)
    b1cat[0, 128:256] = bl1 + br1          # fd side carries both linear biases
    W2cat = np.concatenate([Wl2, Wr2], axis=1).astype(ml_dtypes.bfloat16)
    b2cat = np.zeros((1, 128), np.float32)
    b2cat[0, 64:128] = bl2 + br2
    b2cat = b2cat.astype(ml_dtypes.bfloat16)
    attn1_rep = np.tile(attn1.reshape(1, -1), (P, 1)).astype(ml_dtypes.bfloat16)
    attn2_rep = np.tile(attn2.reshape(1, -1), (P, 1)).astype(ml_dtypes.bfloat16)
    bl1_rep = np.tile(bl1.reshape(1, -1), (P, 1)).astype(ml_dtypes.bfloat16)
    ident_bf = np.eye(P, dtype=ml_dtypes.bfloat16)
    ones_f32 = np.ones((1, P), np.float32)
    ones_bf = np.ones((1, P), ml_dtypes.bfloat16)

    in_maps = []
    deg_order = []
    for c in range(NC):
        es, ed, deg, order, pos_of, _ = cores[c]
        # slot assignment: edge -> (group, rep, lane)
        p_e = pos_of[ed]                        # sorted position of dst
        g_e = p_e // P
        lane = p_e % P
        o = np.argsort(p_e, kind="stable")
        p_s, g_s, lane_s, es_s = p_e[o], g_e[o], lane[o], es[o]
        # rank within dst
        cnt = np.bincount(p_s, minlength=NPAD)
        run_start = np.concatenate([[0], np.cumsum(cnt)[:-1]])
        rank = np.arange(es_s.shape[0]) - run_start[p_s]
        tile_i = np.asarray(cfg.offs)[g_s] + rank
        slot = tile_i * P + lane_s

        pg = pgid[es_s]
        idx = np.zeros(TT * P, np.int64)
        m0 = np.zeros(TT * P, np.float32)
        m1 = np.zeros(TT * P, np.float32)
        msk = np.zeros(TT * P, np.float32)
        idx[slot] = pg // 2
        par = (pg & 1).astype(np.float32)
        m0[slot] = 1.0 - par
        m1[slot] = par
        msk[slot] = 1.0

        # permuted, transposed node features
        xT = np.zeros((cfg.F_IN, NPAD), np.float32)
        xT[:, :NLOC] = x[c * NLOC:(c + 1) * NLOC][order].T

        in_maps.append({
            "xT": xT,
            "W1cat": W1cat, "b1cat": b1cat,
            "W2cat": np.asarray(W2cat), "b2cat": np.asarray(b2cat),
            "attn1_rep": np.asarray(attn1_rep),
            "attn2_rep": np.asarray(attn2_rep),
            "bl1_rep": np.asarray(bl1_rep),
            "ident_bf": np.asarray(ident_bf),
            "ones_f32": ones_f32, "ones_bf": np.asarray(ones_bf),
            "idx": _wrap_idx(idx.astype(np.int16)),
            "m1i": _slot_cols(m1, np.int8),
            "maskt": _slot_cols(msk, ml_dtypes.bfloat16),
        })
        deg_order.append((deg, order))
    cfg._deg_order = deg_order
    cfg._bl2 = bl2
    return in_maps


def _gather_batches(cfg):
    """Partition groups into consecutive batches of <= SG_TILES tiles.
    The trailing groups get small batches so the compute tail after the
    last gather (which blocks the next collective) stays short."""
    batches = []
    cur = []
    cur_t = 0
    tail_tiles = sum(cfg.NT[-8:])
    tail_start = cfg.NG - 8
    for g in range(cfg.NG):
        cap = SG_TILES if g < tail_start else max(SG_TILES // 3, 12)
        t = cfg.NT[g]
        if cur and cur_t + t > cap:
            batches.append(cur)
            cur, cur_t = [], 0
        cur.append(g)
        cur_t += t
    if cur:
        batches.append(cur)
    return batches


def build_program(cfg: Cfg, debug=False):
    nc = bacc.Bacc("TRN2", target_bir_lowering=False, debug=debug,
                   num_devices=cfg.NC)
    NG, NPAD, NLOC, TT = cfg.NG, cfg.NPAD, cfg.NLOC, cfg.TT
    F_IN, D1TOT, H1, D1, OUT = cfg.F_IN, cfg.D1TOT, cfg.H1, cfg.D1, cfg.OUT
    offs = cfg.offs
    core_ids = list(range(cfg.NC))

    par = {}

    def param(name, shape, dtype):
        par[name] = nc.declare_dram_parameter(name, list(shape), dtype,
                                              isOutput=False)
        return par[name]

    xT = param("xT", (F_IN, NPAD), F32)
    W1cat = param("W1cat", (F_IN, 2 * D1TOT), F32)
    b1cat = param("b1cat", (1, 2 * D1TOT), F32)
    W2cat = param("W2cat", (D1TOT, 2 * OUT), BF16)
    b2cat = param("b2cat", (1, 2 * OUT), BF16)
    attn1_rep = param("attn1_rep", (P, D1TOT), BF16)
    attn2_rep = param("attn2_rep", (P, OUT), BF16)
    bl1_rep = param("bl1_rep", (P, D1TOT), BF16)
    ident_bf = param("ident_bf", (P, P), BF16)
    ones_f32 = param("ones_f32", (1, P), F32)
    ones_bf = param("ones_bf", (1, P), BF16)
    idx = param("idx", (P, TT * 8), I16)
    m1i = param("m1i", (P, TT), mybir.dt.int8)
    maskt = param("maskt", (P, TT), BF16)

    out_local = nc.declare_dram_parameter("out_local", [NPAD, OUT], F32,
                                          isOutput=True)

    fs1_local = nc.dram_tensor("fs1_local", [NPAD, D1TOT], BF16)
    fs1_full = nc.dram_tensor("fs1_full", [cfg.N8 // 2, 2 * D1TOT], BF16,
                              addr_space="Shared")
    fs2_local = nc.dram_tensor("fs2_local", [NPAD, 2 * OUT], BF16)
    fs2_full = nc.dram_tensor("fs2_full", [cfg.N8 // 2, 4 * OUT], BF16,
                              addr_space="Shared")

    batches = _gather_batches(cfg)

    with tile.TileContext(nc) as tc:
        with tc.tile_pool(name="const", bufs=1) as cpool:
            c_attn1 = cpool.tile([P, D1TOT], BF16)
            nc.sync.dma_start(out=c_attn1[:], in_=attn1_rep[:, :])
            c_attn2 = cpool.tile([P, OUT], BF16)
            nc.sync.dma_start(out=c_attn2[:], in_=attn2_rep[:, :])
            c_bl1 = cpool.tile([P, D1TOT], BF16)
            nc.sync.dma_start(out=c_bl1[:], in_=bl1_rep[:, :])
            c_ident = cpool.tile([P, P], BF16)
            nc.sync.dma_start(out=c_ident[:], in_=ident_bf[:, :])
            c_ones_f = cpool.tile([1, P], F32)
            nc.sync.dma_start(out=c_ones_f[:], in_=ones_f32[:, :])
            c_ones_bf = cpool.tile([1, P], BF16)
            nc.sync.dma_start(out=c_ones_bf[:], in_=ones_bf[:, :])
            c_W2 = cpool.tile([D1TOT, 2 * OUT], BF16)
            nc.sync.dma_start(out=c_W2[:], in_=W2cat[:, :])
            c_b2 = cpool.tile([1, 2 * OUT], BF16)
            nc.sync.dma_start(out=c_b2[:], in_=b2cat[:, :])
            c_idx = cpool.tile([P, TT * 8], I16)
            nc.sync.dma_start(out=c_idx[:], in_=idx[:, :])
            c_m1 = cpool.tile([P, TT], mybir.dt.int8)
            nc.sync.dma_start(out=c_m1[:], in_=m1i[:, :])
            c_msk = cpool.tile([P, TT], BF16)
            nc.sync.dma_start(out=c_msk[:], in_=maskt[:, :])
            c_fd1 = cpool.tile([P, NG * D1TOT], BF16)
            c_fd2 = cpool.tile([P, NG * OUT], BF16)

            # ================= phase A: layer-1 node tables =================
            with (
                tc.tile_pool(name="ph0", bufs=1) as p0,
                tc.tile_pool(name="ph0w", bufs=3) as p0w,
                tc.tile_pool(name="psA", bufs=2, space="PSUM") as psA,
            ):
                c_W1 = p0.tile([F_IN, 2 * D1TOT], F32)
                nc.sync.dma_start(out=c_W1[:], in_=W1cat[:, :])
                c_b1 = p0.tile([1, 2 * D1TOT], F32)
                nc.sync.dma_start(out=c_b1[:], in_=b1cat[:, :])
                c_xT = p0.tile([F_IN, NPAD], F32)
                nc.sync.dma_start(out=c_xT[:], in_=xT[:, :])
                # pass 1: fs rows only (feeds the AllGather)
                for g in range(NG):
                    ps = psA.tile([P, D1TOT], F32, tag="psf")
                    nc.tensor.matmul(out=ps[:], lhsT=c_xT[:, g * P:(g + 1) * P],
                                     rhs=c_W1[:, 0:D1TOT], start=True,
                                     stop=True)
                    sb = p0w.tile([P, D1TOT], BF16, tag="t1sb")
                    nc.scalar.activation(
                        out=sb[:], in_=ps[:],
                        func=mybir.ActivationFunctionType.Copy)
                    nc.sync.dma_start(out=fs1_local[g * P:(g + 1) * P, :],
                                      in_=sb[:])

                tc.strict_bb_all_engine_barrier()
                nc.gpsimd.collective_compute(
                    "AllGather", mybir.AluOpType.bypass,
                    replica_groups=[core_ids],
                    ins=[fs1_local[:, :]], outs=[fs1_full[:, :]],
                )
                # pass 2: fd rows (local-only) run while the collective is
                # in flight
                for g in range(NG):
                    ps = psA.tile([P, D1TOT], F32, tag="psd")
                    nc.tensor.matmul(out=ps[:], lhsT=c_xT[:, g * P:(g + 1) * P],
                                     rhs=c_W1[:, D1TOT:2 * D1TOT], start=True,
                                     stop=False)
                    nc.tensor.matmul(out=ps[:], lhsT=c_ones_f[:],
                                     rhs=c_b1[:, D1TOT:2 * D1TOT],
                                     start=False, stop=True)
                    nc.scalar.activation(
                        out=c_fd1[:, g * D1TOT:(g + 1) * D1TOT],
                        in_=ps[:],
                        func=mybir.ActivationFunctionType.Copy)
            tc.strict_bb_all_engine_barrier()

            # ============== phase B: layer-1 edges + layer-2 tables =========
            _edge_phase(
                nc, tc, cfg, layer=1, table=fs1_full,
                c_idx=c_idx, c_m1=c_m1, c_msk=c_msk,
                c_fd=c_fd1, c_attn=c_attn1, feat=D1TOT, nheads=H1, hdim=D1,
                batches=batches,
                c_bl1=c_bl1, c_ident=c_ident, c_W2=c_W2, c_b2=c_b2,
                c_ones_bf=c_ones_bf, c_fd2=c_fd2,
                fs2_local=fs2_local, out_local=None,
            )

            tc.strict_bb_all_engine_barrier()
            nc.gpsimd.collective_compute(
                "AllGather", mybir.AluOpType.bypass,
                replica_groups=[core_ids],
                ins=[fs2_local[:, :]], outs=[fs2_full[:, :]],
            )
            tc.strict_bb_all_engine_barrier()

            # ================= phase D: layer-2 edges =======================
            _edge_phase(
                nc, tc, cfg, layer=2, table=fs2_full,
                c_idx=c_idx, c_m1=c_m1, c_msk=c_msk,
                c_fd=c_fd2, c_attn=c_attn2, feat=OUT, nheads=1, hdim=OUT,
                batches=batches,
                c_bl1=None, c_ident=None, c_W2=None, c_b2=None,
                c_ones_bf=None, c_fd2=None,
                fs2_local=None, out_local=out_local,
            )

    nc.compile()
    return nc


def _edge_phase(nc, tc, cfg, layer, table,
                c_idx, c_m1, c_msk, c_fd, c_attn,
                feat, nheads, hdim, batches,
                c_bl1, c_ident, c_W2, c_b2, c_ones_bf, c_fd2,
                fs2_local, out_local):
    offs = cfg.offs
    OUT = cfg.OUT
    acols = feat + nheads
    # pair-row layout: row j of `table` = [node(2j) | node(2j+1)], each `row_w`
    # wide, of which the first `feat` cols are the fs features we use.
    row_w = cfg.D1TOT if layer == 1 else 2 * OUT

    def gather_chunked(out_tile, idx_col0, n_idx):
        done = 0
        while done < n_idx:
            n = min(MAXI, n_idx - done)
            nc.gpsimd.dma_gather(
                out_ap=out_tile[:, (done // P):((done + n) // P), :],
                in_ap=table[:, :],
                idxs_ap=c_idx[:, idx_col0 + done // 16:idx_col0 + (done + n) // 16],
                num_idxs=n, num_idxs_reg=n, elem_size=2 * row_w)
            done += n

    NTMAX = max(cfg.NT)
    with (
        tc.tile_pool(name=f"g{layer}", bufs=2) as gp,
        tc.tile_pool(name=f"wk{layer}", bufs=3) as wk,
        tc.tile_pool(name=f"sm{layer}", bufs=4) as sm,
        tc.tile_pool(name=f"ac{layer}", bufs=2) as ap_,
        tc.tile_pool(name=f"ps{layer}", bufs=2, space="PSUM") as pp,
    ):
        for batch in batches:
            t0, t1_ = offs[batch[0]], offs[batch[-1] + 1]
            nt_b = t1_ - t0
            gbuf = gp.tile([P, nt_b, 2 * row_w], BF16, tag="g")
            gather_chunked(gbuf, t0 * 8, nt_b * P)

            fsb = gp.tile([P, nt_b, feat], BF16, tag="fsb")
            nc.vector.tensor_copy(out=fsb[:], in_=gbuf[:, :, 0:feat])
            nc.vector.copy_predicated(
                out=fsb[:],
                mask=c_m1[:, t0:t1_].unsqueeze(2).to_broadcast(
                    [P, nt_b, feat]),
                data=gbuf[:, :, row_w:row_w + feat])

            for g in batch:
                ng_t = offs[g + 1] - offs[g]
                wbuf = ap_.tile([P, NTMAX, feat], BF16, tag="wb")
                lgb = ap_.tile([P, NTMAX * nheads], F32, tag="lgb")
                fd_g = c_fd[:, g * feat:(g + 1) * feat]
                for c0 in range(offs[g], offs[g + 1], CH):
                    nt = min(CH, offs[g + 1] - c0)
                    w0 = c0 - offs[g]
                    fs = fsb[:, c0 - t0:c0 - t0 + nt, :]
                    u = wk.tile([P, CH, feat], BF16, tag="u")
                    nc.vector.tensor_tensor(
                        out=u[:, 0:nt, :], in0=fs,
                        in1=fd_g.unsqueeze(1).to_broadcast([P, nt, feat]),
                        op=mybir.AluOpType.add)
                    t_lr = wk.tile([P, CH, feat], BF16, tag="tlr")
                    uf = u[:, 0:nt, :]
                    nc.vector.scalar_tensor_tensor(
                        out=t_lr[:, 0:nt, :], in0=uf, scalar=NEG_SLOPE,
                        in1=uf, op0=mybir.AluOpType.mult,
                        op1=mybir.AluOpType.max)
                    tp = wk.tile([P, CH, feat], BF16, tag="tp")
                    nc.vector.tensor_tensor(
                        out=tp[:, 0:nt, :], in0=t_lr[:, 0:nt, :],
                        in1=c_attn[:].unsqueeze(1).to_broadcast([P, nt, feat]),
                        op=mybir.AluOpType.mult)
                    nc.vector.tensor_reduce(
                        out=lgb[:, w0 * nheads:(w0 + nt) * nheads].rearrange(
                            "p (n h) -> p n h", n=nt),
                        in_=tp[:, 0:nt, :].rearrange(
                            "p n (h d) -> p n h d", h=nheads),
                        axis=mybir.AxisListType.X, op=mybir.AluOpType.add)

                # ---- per-group: softmax numerators + reduce + normalize ----
                a_g = sm.tile([P, NTMAX * nheads], BF16, tag="a")
                nc.scalar.activation(
                    out=a_g[:, 0:ng_t * nheads], in_=lgb[:, 0:ng_t * nheads],
                    func=mybir.ActivationFunctionType.Exp)
                am_g = sm.tile([P, NTMAX * nheads], BF16, tag="am")
                nc.vector.tensor_tensor(
                    out=am_g[:, 0:ng_t * nheads].rearrange(
                        "p (n h) -> p n h", n=ng_t),
                    in0=a_g[:, 0:ng_t * nheads].rearrange(
                        "p (n h) -> p n h", n=ng_t),
                    in1=c_msk[:, offs[g]:offs[g + 1]].unsqueeze(2)
                        .to_broadcast([P, ng_t, nheads]),
                    op=mybir.AluOpType.mult)
                nc.vector.tensor_tensor(
                    out=wbuf[:, 0:ng_t, :].rearrange(
                        "p n (h d) -> p n h d", h=nheads),
                    in0=fsb[:, offs[g] - t0:offs[g] - t0 + ng_t, :].rearrange(
                        "p n (h d) -> p n h d", h=nheads),
                    in1=am_g[:, 0:ng_t * nheads].rearrange(
                        "p (n h) -> p n h", n=ng_t).unsqueeze(3)
                        .to_broadcast([P, ng_t, nheads, hdim]),
                    op=mybir.AluOpType.mult)
                acc = sm.tile([P, acols], F32, tag="acc")
                nc.vector.tensor_reduce(
                    out=acc[:, 0:feat],
                    in_=wbuf[:, 0:ng_t, :].rearrange("p n f -> p f n"),
                    axis=mybir.AxisListType.X, op=mybir.AluOpType.add)
                nc.vector.tensor_reduce(
                    out=acc[:, feat:acols],
                    in_=am_g[:, 0:ng_t * nheads].rearrange(
                        "p (n h) -> p h n", n=ng_t),
                    axis=mybir.AxisListType.X, op=mybir.AluOpType.add)
                den0 = acc[:, feat:acols]
                if layer == 1:
                    # fold bl1 back in: acc_w += den0 * bl1 (per head)
                    bt = sm.tile([P, feat], F32, tag="bt")
                    nc.vector.tensor_tensor(
                        out=bt[:].rearrange("p (h d) -> p h d", h=nheads),
                        in0=c_bl1[:].rearrange("p (h d) -> p h d", h=nheads),
                        in1=den0.unsqueeze(2).to_broadcast([P, nheads, hdim]),
                        op=mybir.AluOpType.mult)
                    nc.vector.tensor_tensor(
                        out=acc[:, 0:feat], in0=acc[:, 0:feat], in1=bt[:],
                        op=mybir.AluOpType.add)
                den = sm.tile([P, nheads], F32, tag="den")
                nc.vector.tensor_scalar_add(
                    out=den[:], in0=den0, scalar1=DEN_EPS)
                denr = sm.tile([P, nheads], F32, tag="denr")
                nc.vector.reciprocal(out=denr[:], in_=den[:])
                if layer == 1:
                    h_g = wk.tile([P, feat], BF16, tag="hg")
                    nc.vector.scalar_tensor_tensor(
                        out=h_g[:].rearrange("p (h d) -> p h d", h=nheads),
                        in0=acc[:, 0:feat].rearrange("p (h d) -> p h d",
                                                     h=nheads),
                        scalar=0.0, op0=mybir.AluOpType.max,
                        in1=denr[:].unsqueeze(2).to_broadcast(
                            [P, nheads, hdim]),
                        op1=mybir.AluOpType.mult)
                    ps_t = pp.tile([P, P], BF16, tag="pst")
                    nc.tensor.transpose(out=ps_t[:], in_=h_g[:],
                                        identity=c_ident[:])
                    hT = wk.tile([P, P], BF16, tag="hT")
                    nc.scalar.activation(
                        out=hT[:], in_=ps_t[:],
                        func=mybir.ActivationFunctionType.Copy)
                    ps2 = pp.tile([P, 2 * OUT], F32, tag="ps2")
                    nc.tensor.matmul(out=ps2[:], lhsT=hT[:], rhs=c_W2[:],
                                     start=True, stop=False)
                    nc.tensor.matmul(out=ps2[:], lhsT=c_ones_bf[:],
                                     rhs=c_b2[:], start=False, stop=True)
                    sb2 = wk.tile([P, 2 * OUT], BF16, tag="sb2")
                    nc.scalar.activation(
                        out=sb2[:], in_=ps2[:],
                        func=mybir.ActivationFunctionType.Copy)
                    nc.scalar.activation(
                        out=c_fd2[:, g * OUT:(g + 1) * OUT],
                        in_=sb2[:, OUT:2 * OUT],
                        func=mybir.ActivationFunctionType.Copy)
                    nc.sync.dma_start(out=fs2_local[g * P:(g + 1) * P, :],
                                      in_=sb2[:])
                else:
                    o_g = wk.tile([P, feat], F32, tag="og")
                    nc.vector.tensor_tensor(
                        out=o_g[:], in0=acc[:, 0:feat],
                        in1=denr[:].to_broadcast([P, feat]),
                        op=mybir.AluOpType.mult)
                    nc.sync.dma_start(out=out_local[g * P:(g + 1) * P, :],
                                      in_=o_g[:])


def assemble(results, cfg: Cfg):
    out = np.empty((cfg.N, cfg.OUT), np.float32)
    bl2 = cfg._bl2
    for c in range(cfg.NC):
        loc = np.asarray(results[c]["out_local"])[:cfg.NLOC]   # sorted order
        deg, order = cfg._deg_order[c]
        blk = np.empty((cfg.NLOC, cfg.OUT), np.float32)
        blk[order] = loc
        blk[deg > 0] += bl2[None, :]
        out[c * cfg.NLOC:(c + 1) * cfg.NLOC] = blk
    return out


def kernel(**inputs):
    from concourse.bass_utils import run_bass_kernel_spmd
    cfg = Cfg()
    in_maps = preprocess(inputs, cfg)
    nc = build_program(cfg, debug=not axon_active())
    res = run_bass_kernel_spmd(nc, in_maps, list(range(cfg.NC)))
    return assemble(res.results, cfg)
